# revision 20
# baseline (speedup 1.0000x reference)
"""Trainium2 Bass kernel for a linear-attention transformer block (fp8).

Model (see reference):
  ln1 -> q/k/v proj -> feature map elu(x)+1 -> linear attention via
  per-head kv summary [d,e] and k-sum [d] -> out proj -> residual ->
  ln2 -> MLP (gelu-tanh) -> residual.

Sharding (8 cores): token-parallel. Core c owns batch c//2, sequence half
c%2 (2048 tokens). Everything is token-local except the attention kv
summary (sum over the full sequence of a batch), which is reduced with a
pairwise AllReduce of a [128, 520] bf16 buffer (16 heads x [64, 65]
(kv | ksum), packed two heads per 128 partitions).

Device layout notes:
 - All six big GEMMs (q/k/v/o/fc/proj) run in fp8e4 with the DoubleRow
   perf mode: two 128-deep contraction tiles per matmul pass, i.e. 2x the
   bf16 matmul throughput.  Weights are pre-scaled by powers of two on the
   host so their distributions sit in fp8's sweet spot; the inverse scale
   is folded into the activation/copy op that drains each PSUM.
 - fp8 weights are small enough (12 MB total) to stay resident in SBUF,
   so nothing is re-streamed from HBM and the ln1(x).T intermediate also
   stays resident (no DRAM spill).
 - Activations are token-major [128 tokens, features]; matmuls contract
   over the partition axis, so activation tiles are PE-transposed where
   a matmul needs them feature-major, staged 4 chunks per PSUM bank with
   a single strided copy out.
 - The kv summary accumulates over the whole pass directly in PSUM
   (one long matmul accumulation group per head), not via vector adds.
 - The per-token attention normalizer is applied with a stride-0
   broadcast tensor_tensor against the reciprocal of the strided
   denominator columns read straight out of the apply PSUM.
"""

import os
import sys
from contextlib import ExitStack

import numpy as np

for _p in ("/opt/trn_rl_repo",):
    if _p not in sys.path:
        sys.path.insert(0, _p)

import ml_dtypes  # noqa: E402

import concourse.bass as bass  # noqa: E402
import concourse.tile as tile  # noqa: E402
from concourse import bacc  # noqa: E402
from concourse import mybir  # noqa: E402
from concourse.masks import make_identity  # noqa: E402

BF16 = mybir.dt.bfloat16
FP32 = mybir.dt.float32
FP8 = mybir.dt.float8e4
AF = mybir.ActivationFunctionType
ALU = mybir.AluOpType
DR = mybir.MatmulPerfMode.DoubleRow

# Model dims (fixed by the problem).
B, S, H = 4, 4096, 1024
NH, HD = 16, 64
MLP = 4096

HC = H // 128    # 8 contraction chunks over hidden dim
FO = H // 128    # 8 feature chunks (q feature-major)
MO = MLP // 128  # 32 mlp chunks
BLK = 512        # tokens per block
TS = BLK // 128  # 128-token subtiles per block

LN_EPS = 1e-5

# fp8 weight pre-scales (powers of two; inverse folded into psum drains).
SQKV = 16.0      # q/k/v weights (sigma 0.02 -> 0.32)
SFC = 16.0       # fc weights
SPJ = 64.0       # proj weights (sigma 0.004 -> 0.26)
SKV = 1.0 / 32.0  # kv-summary scale into fp8 kvaug (ksum ~4300 -> ~134)


def build_kernel(nc, t_core, n_cores, apply_bias=False):
    """Emit the per-core program."""
    T = t_core
    nblk = T // BLK
    groups = [[2 * i, 2 * i + 1] for i in range(n_cores // 2)]

    x_d = nc.dram_tensor("x", [T, H], FP32, kind="ExternalInput")
    qw_d = nc.dram_tensor("qw", [128, HC * H], FP8, kind="ExternalInput")
    kw_d = nc.dram_tensor("kw", [128, HC * H], FP8, kind="ExternalInput")
    vw_d = nc.dram_tensor("vw", [128, HC * H], FP8, kind="ExternalInput")
    ow_d = nc.dram_tensor("ow", [128, HC * H], FP8, kind="ExternalInput")
    fcw_d = nc.dram_tensor("fcw", [128, MO * H], FP8, kind="ExternalInput")
    pjw_d = nc.dram_tensor("projw", [128, MO * H], FP8, kind="ExternalInput")
    bias_d = {}
    if apply_bias:
        for nm, n in (("qb", H), ("kb", H), ("vb", H), ("ob", H),
                      ("fcb", MLP), ("projb", H)):
            bias_d[nm] = nc.dram_tensor(nm, [1, n], BF16, kind="ExternalInput")
    out_d = nc.dram_tensor("out", [T, H], FP32, kind="ExternalOutput")

    with tile.TileContext(nc) as tc, ExitStack() as ctx:
        consts = ctx.enter_context(tc.tile_pool(name="consts", bufs=1))
        wpool = ctx.enter_context(tc.tile_pool(name="wpool", bufs=1))
        acts = ctx.enter_context(tc.tile_pool(name="acts", bufs=2))
        dram = ctx.enter_context(tc.tile_pool(name="dram", bufs=1, space="DRAM"))
        # PSUM budget (8 banks):
        #   big   2 x [128,1024] f32 (4 banks): k/v/q/o psums, proj accum
        #   mid   2 x [128, 512]      (2 banks): transpose staging + fc psum
        #   small 2 x [128, 260] f32 (2 banks): kv-summary accum, apply
        psum = ctx.enter_context(tc.tile_pool(name="psum", bufs=2, space="PSUM"))

        # ---- constants ----------------------------------------------------
        ident = consts.tile([128, 128], BF16)
        make_identity(nc, ident)
        ident_f8 = consts.tile([128, 128], FP8)
        nc.vector.tensor_copy(ident_f8, ident)
        eps_ln = consts.tile([128, 1], FP32)
        nc.vector.memset(eps_ln, LN_EPS)
        if apply_bias:
            ones_row = consts.tile([1, 128], BF16)
            nc.vector.memset(ones_row, 1.0)
            ones_t = consts.tile([1, BLK], BF16)
            nc.vector.memset(ones_t, 1.0)
            bias_sb = {}
            for nm, n in (("qb", H), ("kb", H), ("vb", H), ("ob", H),
                          ("fcb", MLP), ("projb", H)):
                b_t = consts.tile([1, n], BF16, name=f"{nm}_sb")
                nc.sync.dma_start(out=b_t, in_=bias_d[nm][:, :])
                bias_sb[nm] = b_t

        # resident fp8 weights (12 MB total); k/v load first (needed first)
        kw = wpool.tile([128, HC * H], FP8)
        nc.sync.dma_start(out=kw, in_=kw_d[:, :])
        vw = wpool.tile([128, HC * H], FP8)
        nc.sync.dma_start(out=vw, in_=vw_d[:, :])
        ow = wpool.tile([128, HC * H], FP8)
        nc.sync.dma_start(out=ow, in_=ow_d[:, :])
        qw = wpool.tile([128, HC * H], FP8)
        nc.sync.dma_start(out=qw, in_=qw_d[:, :])
        fcw = wpool.tile([128, MO * H], FP8)
        nc.sync.dma_start(out=fcw, in_=fcw_d[:, :])
        pjw = wpool.tile([128, MO * H], FP8)
        nc.sync.dma_start(out=pjw, in_=pjw_d[:, :])

        qw3 = qw.rearrange("p (c f) -> p c f", c=HC)   # [p, hc, FO*128]
        kw3 = kw.rearrange("p (c f) -> p c f", c=HC)   # [p, hc, H]
        vw3 = vw.rearrange("p (c f) -> p c f", c=HC)
        ow3 = ow.rearrange("p (c f) -> p c f", c=HC)
        pjw3 = pjw.rearrange("p (m f) -> p m f", m=MO)  # [p, mo, H]

        # ln1(x).T stays resident in fp8: [p, hc*T + t]
        lnxT = wpool.tile([128, HC * T], FP8)
        lnxT3 = lnxT.rearrange("p (c t) -> p c t", c=HC)

        # k/v staging for the DoubleRow kv summary: 2 manual buffers each,
        # indexed by ts-pair parity. vt is padded with a ones column per
        # head (65-stride) so one matmul produces kv and ksum together.
        kfb = [wpool.tile([128, 2 * H], FP8, name=f"kfb{i}") for i in range(2)]
        vtb = [wpool.tile([128, 2 * NH * 65], FP8, name=f"vtb{i}")
               for i in range(2)]
        for i in range(2):
            nc.vector.memset(vtb[i], 1.0)

        # kv-summary PSUM accumulators: heads 0..7 / 8..15, head h at
        # partitions (h%2)*64, cols ((h%8)//2)*65.
        # Full-bank-width tiles (cols 260:512 unused) so partition-base-64
        # writes keep each partition's flat range within one bank.
        kvs = [psum.tile([128, 512], FP32, tag="small", bufs=2, name=f"kvs{i}")
               for i in range(2)]
        for i in range(2):
            nc.vector.memset(kvs[i], 0.0)

        kvaug = consts.tile([128, 8 * 130], FP8)
        nc.vector.memset(kvaug, 0.0)

        def layernorm_to(xt, dst):
            """dst = (xt - mean) * rsqrt(var + eps), cast to dst dtype.

            Stats on vector; the 1024-wide apply runs on the scalar engine
            as Identity(x*rstd - mu*rstd)."""
            stats = acts.tile([128, 2, 6], FP32, tag="ln_stats", bufs=3)
            nc.vector.bn_stats(out=stats[:, 0, :], in_=xt[:, 0:512])
            nc.vector.bn_stats(out=stats[:, 1, :], in_=xt[:, 512:1024])
            mv = acts.tile([128, 2], FP32, tag="ln_mv", bufs=3)
            nc.vector.bn_aggr(out=mv, in_=stats)
            rstd = acts.tile([128, 1], FP32, tag="ln_rstd", bufs=3)
            nc.scalar.activation(out=rstd, in_=mv[:, 1:2], func=AF.Sqrt,
                                 bias=eps_ln, scale=1.0)
            nc.vector.reciprocal(out=rstd, in_=rstd)
            mb = acts.tile([128, 1], FP32, tag="ln_mb", bufs=3)
            nc.vector.tensor_scalar(out=mb, in0=mv[:, 0:1], scalar1=rstd[:, 0:1],
                                    scalar2=-1.0, op0=ALU.mult, op1=ALU.mult)
            nc.scalar.activation(out=dst, in_=xt, func=AF.Identity,
                                 bias=mb[:, 0:1], scale=rstd[:, 0:1])

        def transpose_chunks(src_fp8, dstT3, t0):
            """PE-transpose token-major fp8 [128,1024] into 8 chunks of
            dstT3 ([p, c, T] view, fp8), staged 4 chunks per PSUM bank.

            The fp8 transposer emits with element step 2 (16-bit lanes), so
            the staging view interleaves a dead byte per element."""
            for g in range(2):
                pt = psum.tile([128, 1024], FP8, tag="mid", bufs=2, name="pt")
                ptv = pt.rearrange("p (c t w) -> p c t w", c=4, w=2)
                for c in range(4):
                    nc.tensor.transpose(
                        ptv[:, c:c + 1, :, 0:1],
                        src_fp8[:, (g * 4 + c) * 128:(g * 4 + c + 1) * 128],
                        ident_f8)
                dst = dstT3[:, g * 4:g * 4 + 4, t0:t0 + 128] \
                    .rearrange("p c (t w) -> p c t w", w=1)
                src = ptv[:, :, :, 0:1]
                if g == 0:
                    nc.vector.tensor_copy(dst, src)
                else:
                    nc.scalar.copy(out=dst, in_=src)

        def feature_map(ps, dst, n, scale):
            """dst = elu(ps*scale)+1 = min(exp(ps*scale),1) + relu(ps*scale)."""
            e = acts.tile([128, n], BF16, tag="fm_e", bufs=3, name="fm_e")
            nc.scalar.activation(out=e, in_=ps, func=AF.Exp, scale=scale)
            r = acts.tile([128, n], BF16, tag="fm_r", bufs=2, name="fm_r")
            nc.vector.tensor_scalar(out=r, in0=ps, scalar1=0.0, scalar2=scale,
                                    op0=ALU.max, op1=ALU.mult)
            nc.vector.scalar_tensor_tensor(out=dst, in0=e, scalar=1.0, in1=r,
                                           op0=ALU.min, op1=ALU.add)

        def dr_proj(pp, lhsT3, tok0, w3, bias_t=None):
            """[128,1024] psum = x_chunkT.T @ W via fp8 DoubleRow matmuls."""
            for half in range(2):
                for c in range(HC // 2):
                    nc.tensor.matmul(
                        pp[:, half * 512:half * 512 + 512],
                        lhsT=lhsT3[:, 2 * c:2 * c + 2, tok0:tok0 + 128],
                        rhs=w3[:, 2 * c:2 * c + 2, half * 512:half * 512 + 512],
                        start=(c == 0),
                        stop=(c == HC // 2 - 1 and bias_t is None),
                        perf_mode=DR)
                if bias_t is not None:
                    nc.tensor.matmul(
                        pp[:, half * 512:half * 512 + 512], lhsT=ones_row,
                        rhs=bias_t[0:1, half * 512:half * 512 + 512],
                        start=False, stop=True)

        # ================== PASS A: ln1, k/v, kv summary ==================
        for blk in range(nblk):
            for ts in range(TS):
                xt = acts.tile([128, H], FP32, tag="xin", bufs=2)
                r0 = blk * BLK + ts * 128
                nc.gpsimd.dma_start(out=xt, in_=x_d[r0:r0 + 128, :])
                lnx = acts.tile([128, H], FP8, tag="lnx", bufs=3)
                layernorm_to(xt, lnx)
                transpose_chunks(lnx, lnxT3, r0)

            for pair in range(TS // 2):
                kf2 = kfb[pair % 2]
                vt2 = vtb[pair % 2]
                for s in range(2):
                    tok0 = blk * BLK + (pair * 2 + s) * 128
                    kp = psum.tile([128, 1024], FP32, tag="big", bufs=2,
                                   name="kp")
                    dr_proj(kp, lnxT3, tok0, kw3,
                            bias_sb["kb"] if apply_bias else None)
                    feature_map(kp, kf2[:, s * H:(s + 1) * H], H, 1.0 / SQKV)

                    vp = psum.tile([128, 1024], FP32, tag="big", bufs=2,
                                   name="vp")
                    dr_proj(vp, lnxT3, tok0, vw3,
                            bias_sb["vb"] if apply_bias else None)
                    # scaled copy into the 65-strided padded layout
                    src3 = vp.rearrange("p (h c) -> p h c", c=64)
                    dst3 = vt2.rearrange("p (g c) -> p g c", c=65)
                    d = dst3[:, s * NH:s * NH + NH, 0:64]
                    nc.vector.tensor_scalar_mul(out=d[:, 0:8, :],
                                                in0=src3[:, 0:8, :],
                                                scalar1=1.0 / SQKV)
                    nc.scalar.mul(out=d[:, 8:16, :], in_=src3[:, 8:16, :],
                                  mul=1.0 / SQKV)

                # kv summary over this 256-token pair (DoubleRow over the
                # two 128-token subtiles), accumulating in PSUM all pass.
                kf3 = kf2.rearrange("p (s f) -> p s f", s=2)
                vt3 = vt2.rearrange("p (s f) -> p s f", s=2)
                first = (blk == 0 and pair == 0)
                last = (blk == nblk - 1 and pair == TS // 2 - 1)
                # Accumulate onto the pre-zeroed PSUM with start=False
                # (plain read-modify-write); the per-bank group tracker
                # can't express 4 packed head-pair regions, so skip it.
                # Even heads (dst partitions 0:64) use DoubleRow; odd heads
                # land at partition base 64, which the DR ISA encoding
                # disallows, so they run as two plain fp8 matmuls.
                for h in range(NH):
                    tl = kvs[h // 8]
                    hh = h % 8
                    dst = tl[(hh % 2) * 64:(hh % 2) * 64 + 64,
                             (hh // 2) * 65:(hh // 2) * 65 + 65]
                    if hh % 2 == 0:
                        nc.tensor.matmul(
                            dst,
                            lhsT=kf3[:, :, h * 64:h * 64 + 64],
                            rhs=vt3[:, :, h * 65:h * 65 + 65],
                            start=False, stop=(last and hh >= 6),
                            skip_group_check=True, perf_mode=DR)
                    else:
                        for s in range(2):
                            nc.tensor.matmul(
                                dst,
                                lhsT=kf2[:, s * H + h * 64:s * H + h * 64 + 64],
                                rhs=vt2[:, s * NH * 65 + h * 65:
                                        s * NH * 65 + h * 65 + 65],
                                start=False,
                                stop=(last and hh >= 6 and s == 1),
                                skip_group_check=True)

        # ================== AllReduce of kv summary over the seq pair =====
        kv_sb = consts.tile([128, 520], BF16)
        nc.vector.tensor_copy(kv_sb[:, 0:260], kvs[0][:, 0:260])
        nc.scalar.copy(out=kv_sb[:, 260:520], in_=kvs[1][:, 0:260])
        cc_in = dram.tile([128, 520], BF16)
        cc_out = dram.tile([128, 520], BF16)
        nc.gpsimd.dma_start(out=cc_in, in_=kv_sb)
        nc.gpsimd.collective_compute(
            "AllReduce", ALU.add, replica_groups=groups,
            ins=[cc_in.opt()], outs=[cc_out.opt()])
        kvred = consts.tile([128, 520], BF16)
        nc.gpsimd.dma_start(out=kvred, in_=cc_out)

        # q projections: blocks 0/1 run while the collective is in
        # flight; later blocks pipeline in behind the attention prologue
        # and MLP tails.
        qfTs = {}

        def q_proj_stages(blk):
            qfT = acts.tile([128, FO * BLK], FP8, tag="qfT", bufs=4,
                            name="qfT")
            qfTs[blk] = qfT

            def piece(fp):
                qp = psum.tile([128, 1024], FP32, tag="big", bufs=2,
                               name="qp")
                for j in range(2):
                    fo = fp * 2 + j
                    for c in range(HC // 2):
                        nc.tensor.matmul(
                            qp[:, j * 512:j * 512 + 512],
                            lhsT=qw3[:, 2 * c:2 * c + 2,
                                     fo * 128:fo * 128 + 128],
                            rhs=lnxT3[:, 2 * c:2 * c + 2,
                                      blk * BLK:blk * BLK + BLK],
                            start=(c == 0),
                            stop=(c == HC // 2 - 1 and not apply_bias),
                            perf_mode=DR)
                    if apply_bias:
                        nc.tensor.matmul(
                            qp[:, j * 512:j * 512 + 512],
                            lhsT=bias_sb["qb"][0:1, fo * 128:fo * 128 + 128],
                            rhs=ones_t[0:1, 0:BLK], start=False, stop=True)
                feature_map(qp, qfT[:, fp * 1024:fp * 1024 + 1024], 1024,
                            1.0 / SQKV)
            return [(lambda f=fp: piece(f)) for fp in range(FO // 2)]

        for _b in range(nblk):
            for _f in q_proj_stages(_b):
                _f()

        # Block-diagonal fp8 [d, (kv|ksum)] pairs, scaled by 1/32.
        kvr3 = kvred.rearrange("p (g c) -> p g c", c=65)
        kva3 = kvaug.rearrange("p (g c) -> p g c", c=130)
        nc.vector.tensor_scalar_mul(out=kva3[0:64, :, 0:65],
                                    in0=kvr3[0:64, :, :], scalar1=SKV)
        nc.scalar.mul(out=kva3[64:128, :, 65:130], in_=kvr3[64:128, :, :],
                      mul=SKV)

        # ================== PASS B: apply, o-proj, residual, MLP ==========
        # Software-pipelined: block N's MLP (PE-dense, vector-idle) runs
        # concurrently with block N+1's attention stage (vector-heavy,
        # PE-light) by interleaving instruction emission. The attention
        # stage therefore avoids the "big" PSUM tag (held by the MLP
        # accumulators): o-proj runs as two [128,512] halves on "small".
        blk_state = {}

        def attn_stages(blk):
            """Closures for one block's attention work: per ts, A1 (apply
            + normalize), A2 (attnT transposes + o-proj + residual), A3
            (LN2 + ln2T transposes)."""
            qfT = qfTs.pop(blk)
            attnT = acts.tile([128, HC * BLK], FP8, tag="attnT", bufs=2)
            attnT3 = attnT.rearrange("p (c t) -> p c t", c=HC)
            ln2T = acts.tile([128, HC * BLK], FP8, tag="ln2T", bufs=2)
            ln2T3 = ln2T.rearrange("p (c t) -> p c t", c=HC)
            xrs = [None] * TS
            attns = [None] * TS
            blk_state[blk] = (ln2T3, xrs)

            def a1(ts):
                attn = acts.tile([128, H], FP8, tag="attn", bufs=2)
                attns[ts] = attn
                for wave in range(4):
                    pa = psum.tile([128, 512], FP32, tag="small", bufs=2,
                                   name="pa")
                    for j in range(2):
                        g = wave * 2 + j
                        nc.tensor.matmul(
                            pa[:, j * 130:j * 130 + 130],
                            lhsT=qfT[:, g * BLK + ts * 128:
                                     g * BLK + ts * 128 + 128],
                            rhs=kvaug[:, g * 130:g * 130 + 130],
                            start=(j == 0), stop=(j == 1))
                    pa3 = pa[:, 0:260].rearrange("p (g c) -> p g c", c=65)
                    rc = acts.tile([128, 4], FP32, tag="rc", bufs=4)
                    nc.vector.reciprocal(
                        out=rc,
                        in_=pa3[:, :, 64:65].rearrange("p g c -> p (g c)"))
                    rcb = bass.AP(rc.tensor, rc.offset,
                                  [rc.ap[0], (1, 4), (0, 64)])
                    dst3 = attn.rearrange("p (g c) -> p g c",
                                          c=64)[:, wave * 4:wave * 4 + 4, :]
                    nc.vector.tensor_tensor(out=dst3, in0=pa3[:, :, 0:64],
                                            in1=rcb, op=ALU.mult)

            xts = [None] * TS

            def a2a(ts):
                transpose_chunks(attns[ts], attnT3, ts * 128)
                xt = acts.tile([128, H], FP32, tag="xin", bufs=2, name="xt2")
                xts[ts] = xt
                r0 = blk * BLK + ts * 128
                nc.gpsimd.dma_start(out=xt, in_=x_d[r0:r0 + 128, :])

            def a2b(ts):
                xt = xts[ts]
                xr = acts.tile([128, H], BF16, tag="xr", bufs=2 * TS + 1,
                               name="xr")
                xrs[ts] = xr
                for half in range(2):
                    oph = psum.tile([128, 512], FP32, tag="small", bufs=2,
                                    name="oph")
                    for c in range(HC // 2):
                        nc.tensor.matmul(
                            oph,
                            lhsT=attnT3[:, 2 * c:2 * c + 2,
                                        ts * 128:ts * 128 + 128],
                            rhs=ow3[:, 2 * c:2 * c + 2,
                                    half * 512:half * 512 + 512],
                            start=(c == 0),
                            stop=(c == HC // 2 - 1 and not apply_bias),
                            perf_mode=DR)
                    if apply_bias:
                        nc.tensor.matmul(
                            oph, lhsT=ones_row,
                            rhs=bias_sb["ob"][0:1, half * 512:
                                              half * 512 + 512],
                            start=False, stop=True)
                    nc.vector.tensor_add(
                        out=xr[:, half * 512:half * 512 + 512],
                        in0=xt[:, half * 512:half * 512 + 512], in1=oph)

            ln2s = [None] * TS

            def a3a(ts):
                ln2 = acts.tile([128, H], FP8, tag="lnx", bufs=3, name="ln2")
                ln2s[ts] = ln2
                layernorm_to(xrs[ts], ln2)

            def a3b(ts):
                transpose_chunks(ln2s[ts], ln2T3, ts * 128)

            # Stagger the per-ts chains so every cross-engine dependency
            # has a couple of MLP chunks of slack before the PE needs it.
            sched = []
            for ts in range(TS):
                base = 5 * ts
                for off, fn in ((0, a1), (2, a2a), (4, a2b), (5, a3a),
                                (7, a3b)):
                    sched.append((base + off, ts, fn))
            sched.sort(key=lambda z: (z[0], z[1]))
            out = []
            for _, ts, fn in sched:
                gate = None
                if fn is a3b:
                    if ts == 1:
                        gate = (blk, 0)
                    elif ts == TS - 1:
                        gate = (blk, 1)
                out.append(((lambda t=ts, f=fn: f(t)), gate))
            return out

        from collections import deque
        pending = deque(attn_stages(0))
        gates_done = set()

        def pull_one():
            if pending:
                fn, gate = pending.popleft()
                fn()
                if gate is not None:
                    gates_done.add(gate)

        def drain_until(gate):
            while gate not in gates_done and pending:
                pull_one()

        for blk in range(nblk):
            if blk + 1 < nblk:
                pending.extend(attn_stages(blk + 1))
            for th in range(2):
                drain_until((blk, th))
                ln2T3, xrs = blk_state[blk]
                pps = [psum.tile([128, 1024], FP32, tag="big", bufs=2,
                                 name=f"pps_{th}_{i}") for i in range(2)]
                for mp in range(MO // 2):
                    pfc = psum.tile([128, 512], FP32, tag="mid", bufs=2,
                                    name="pfc")
                    for j in range(2):
                        mo = mp * 2 + j
                        for c in range(HC // 2):
                            nc.tensor.matmul(
                                pfc[:, j * 256:j * 256 + 256],
                                lhsT=fcw[:, mo * 1024 + 2 * c * 128:
                                         mo * 1024 + 2 * c * 128 + 256]
                                    .rearrange("p (k m) -> p k m", k=2),
                                rhs=ln2T3[:, 2 * c:2 * c + 2,
                                          th * 256:th * 256 + 256],
                                start=(j == 0 and c == 0),
                                stop=(j == 1 and c == HC // 2 - 1
                                      and not apply_bias),
                                perf_mode=DR)
                        if apply_bias:
                            nc.tensor.matmul(
                                pfc[:, j * 256:j * 256 + 256],
                                lhsT=bias_sb["fcb"][0:1,
                                                    mo * 128:mo * 128 + 128],
                                rhs=ones_t[0:1, 0:256], start=False,
                                stop=(j == 1))
                    hT2 = acts.tile([128, 512], FP8, tag="hT", bufs=3)
                    nc.scalar.activation(out=hT2, in_=pfc,
                                         func=AF.Gelu_apprx_tanh,
                                         scale=1.0 / SFC)
                    h3 = hT2.rearrange("p (j t) -> p j t", j=2)
                    for tsl in range(2):
                        for half in range(2):
                            nc.tensor.matmul(
                                pps[tsl][:, half * 512:half * 512 + 512],
                                lhsT=h3[:, :, tsl * 128:tsl * 128 + 128],
                                rhs=pjw3[:, mp * 2:mp * 2 + 2,
                                         half * 512:half * 512 + 512],
                                start=(mp == 0),
                                stop=(mp == MO // 2 - 1 and not apply_bias),
                                perf_mode=DR)
                    # pull one pipelined attention sub-stage per chunk
                    pull_one()
                if apply_bias:
                    for tsl in range(2):
                        for half in range(2):
                            nc.tensor.matmul(
                                pps[tsl][:, half * 512:half * 512 + 512],
                                lhsT=ones_row,
                                rhs=bias_sb["projb"][0:1, half * 512:
                                                     half * 512 + 512],
                                start=False, stop=True)
                for tsl in range(2):
                    ts_ = th * 2 + tsl
                    outt = acts.tile([128, H], FP32, tag="outt", bufs=2)
                    nc.vector.scalar_tensor_tensor(
                        out=outt, in0=pps[tsl], scalar=1.0 / SPJ,
                        in1=xrs[ts_], op0=ALU.mult, op1=ALU.add)
                    r0 = blk * BLK + ts_ * 128
                    nc.sync.dma_start(out=out_d[r0:r0 + 128, :], in_=outt)
            blk_state.pop(blk)

# ======================= host side =======================================

def _prep_weights(inputs):
    """Fold LN affine params into adjacent weights; scale + cast to fp8.

    Returns (tensor_dict, apply_bias). If every effective bias is exactly
    zero (true for this model's initialization), the biases are dropped and
    the device program skips the rank-1 bias updates entirely.
    """
    f32 = lambda k: np.asarray(inputs[k], np.float32)
    bf = ml_dtypes.bfloat16
    f8 = ml_dtypes.float8_e4m3

    def to_f8(w):
        return np.clip(w, -240.0, 240.0).astype(f8)

    ln1_w, ln1_b = f32("ln1_w"), f32("ln1_b")
    ln2_w, ln2_b = f32("ln2_w"), f32("ln2_b")

    out = {}
    biases = {}

    def qkv_like(w, b, scale, q_layout):
        we = (ln1_w[:, None] * w) * scale
        be = (b + ln1_b @ w) * scale
        if q_layout:  # [p, (hc*FO+fo)*128+m]
            wd = we.reshape(HC, 128, FO, 128).transpose(1, 0, 2, 3).reshape(128, HC * H)
        else:         # [p, hc*H+m]
            wd = we.reshape(HC, 128, H).transpose(1, 0, 2).reshape(128, HC * H)
        return to_f8(wd), be.reshape(1, -1).astype(bf)

    out["qw"], biases["qb"] = qkv_like(f32("q_w"), f32("q_b"), SQKV, True)
    out["kw"], biases["kb"] = qkv_like(f32("k_w"), f32("k_b"), SQKV, False)
    out["vw"], biases["vb"] = qkv_like(f32("v_w"), f32("v_b"), SQKV, False)

    o_w, o_b = f32("o_w"), f32("o_b")
    out["ow"] = to_f8(o_w.reshape(HC, 128, H).transpose(1, 0, 2).reshape(128, HC * H))
    biases["ob"] = o_b.reshape(1, H).astype(bf)

    fc_w, fc_b = f32("fc_w"), f32("fc_b")
    fce = (ln2_w[:, None] * fc_w) * SFC
    fcbe = (fc_b + ln2_b @ fc_w) * SFC
    # [p, mo*(HC*128) + hc*128 + m]
    out["fcw"] = to_f8(fce.reshape(HC, 128, MO, 128).transpose(1, 2, 0, 3).reshape(128, MO * H))
    biases["fcb"] = fcbe.reshape(1, MLP).astype(bf)

    pj_w, pj_b = f32("proj_w"), f32("proj_b")
    out["projw"] = to_f8((pj_w * SPJ).reshape(MO, 128, H).transpose(1, 0, 2).reshape(128, MO * H))
    biases["projb"] = (pj_b * SPJ).reshape(1, H).astype(bf)

    apply_bias = any(np.any(np.asarray(b, np.float32) != 0.0)
                     for b in biases.values())
    if apply_bias:
        out.update(biases)
    return out, apply_bias


def _run(inputs, trace=False):
    from concourse.bass_utils import run_bass_kernel_spmd

    n_cores = 8
    t_core = B * S // n_cores  # 2048

    x = np.ascontiguousarray(np.asarray(inputs["x"], np.float32))
    wd, apply_bias = _prep_weights(inputs)

    nc = bacc.Bacc(None, num_devices=n_cores, target_bir_lowering=False)
    build_kernel(nc, t_core, n_cores, apply_bias=apply_bias)
    nc.compile()

    half = S // 2
    in_maps = []
    for c in range(n_cores):
        b, sh = c // 2, c % 2
        m = {"x": np.ascontiguousarray(x[b, sh * half:(sh + 1) * half, :])}
        m.update(wd)
        in_maps.append(m)

    res = run_bass_kernel_spmd(nc, in_maps, core_ids=list(range(n_cores)),
                               trace=trace)

    out = np.empty((B, S, H), np.float32)
    for c in range(n_cores):
        b, sh = c // 2, c % 2
        out[b, sh * half:(sh + 1) * half, :] = res.results[c]["out"]
    return out, res


def kernel(**inputs):
    return _run(inputs)[0]


if __name__ == "__main__":
    os.environ.setdefault("BASS_NEVER_TRACE", "1")
    import reference

    inputs = {k: np.asarray(v) for k, v in reference.setup_inputs().items()}
    got = kernel(**inputs)
    exp = np.asarray(reference.reference(**inputs))
    err = np.abs(got - exp).max() / np.abs(exp).max()
    print("Relative error:", err)


# revision 21
# speedup vs baseline: 1.0066x; 1.0066x over previous
"""Trainium2 Bass kernel for a linear-attention transformer block (fp8).

Model (see reference):
  ln1 -> q/k/v proj -> feature map elu(x)+1 -> linear attention via
  per-head kv summary [d,e] and k-sum [d] -> out proj -> residual ->
  ln2 -> MLP (gelu-tanh) -> residual.

Sharding (8 cores): token-parallel. Core c owns batch c//2, sequence half
c%2 (2048 tokens). Everything is token-local except the attention kv
summary (sum over the full sequence of a batch), which is reduced with a
pairwise AllReduce of a [128, 520] bf16 buffer (16 heads x [64, 65]
(kv | ksum), packed two heads per 128 partitions).

Device layout notes:
 - All six big GEMMs (q/k/v/o/fc/proj) run in fp8e4 with the DoubleRow
   perf mode: two 128-deep contraction tiles per matmul pass, i.e. 2x the
   bf16 matmul throughput.  Weights are pre-scaled by powers of two on the
   host so their distributions sit in fp8's sweet spot; the inverse scale
   is folded into the activation/copy op that drains each PSUM.
 - fp8 weights are small enough (12 MB total) to stay resident in SBUF,
   so nothing is re-streamed from HBM and the ln1(x).T intermediate also
   stays resident (no DRAM spill).
 - Activations are token-major [128 tokens, features]; matmuls contract
   over the partition axis, so activation tiles are PE-transposed where
   a matmul needs them feature-major, staged 4 chunks per PSUM bank with
   a single strided copy out.
 - The kv summary accumulates over the whole pass directly in PSUM
   (one long matmul accumulation group per head), not via vector adds.
 - The per-token attention normalizer is applied with a stride-0
   broadcast tensor_tensor against the reciprocal of the strided
   denominator columns read straight out of the apply PSUM.
"""

import os
import sys
from contextlib import ExitStack

import numpy as np

for _p in ("/opt/trn_rl_repo",):
    if _p not in sys.path:
        sys.path.insert(0, _p)

import ml_dtypes  # noqa: E402

import concourse.bass as bass  # noqa: E402
import concourse.tile as tile  # noqa: E402
from concourse import bacc  # noqa: E402
from concourse import mybir  # noqa: E402
from concourse.masks import make_identity  # noqa: E402

BF16 = mybir.dt.bfloat16
FP32 = mybir.dt.float32
FP8 = mybir.dt.float8e4
AF = mybir.ActivationFunctionType
ALU = mybir.AluOpType
DR = mybir.MatmulPerfMode.DoubleRow

# Model dims (fixed by the problem).
B, S, H = 4, 4096, 1024
NH, HD = 16, 64
MLP = 4096

HC = H // 128    # 8 contraction chunks over hidden dim
FO = H // 128    # 8 feature chunks (q feature-major)
MO = MLP // 128  # 32 mlp chunks
BLK = 512        # tokens per block
TS = BLK // 128  # 128-token subtiles per block

LN_EPS = 1e-5

# fp8 weight pre-scales (powers of two; inverse folded into psum drains).
SQKV = 16.0      # q/k/v weights (sigma 0.02 -> 0.32)
SFC = 16.0       # fc weights
SPJ = 64.0       # proj weights (sigma 0.004 -> 0.26)
SKV = 1.0 / 32.0  # kv-summary scale into fp8 kvaug (ksum ~4300 -> ~134)


def build_kernel(nc, t_core, n_cores, apply_bias=False):
    """Emit the per-core program."""
    T = t_core
    nblk = T // BLK
    groups = [[2 * i, 2 * i + 1] for i in range(n_cores // 2)]

    x_d = nc.dram_tensor("x", [T, H], FP32, kind="ExternalInput")
    qw_d = nc.dram_tensor("qw", [128, HC * H], FP8, kind="ExternalInput")
    kw_d = nc.dram_tensor("kw", [128, HC * H], FP8, kind="ExternalInput")
    vw_d = nc.dram_tensor("vw", [128, HC * H], FP8, kind="ExternalInput")
    ow_d = nc.dram_tensor("ow", [128, HC * H], FP8, kind="ExternalInput")
    fcw_d = nc.dram_tensor("fcw", [128, MO * H], FP8, kind="ExternalInput")
    pjw_d = nc.dram_tensor("projw", [128, MO * H], FP8, kind="ExternalInput")
    bias_d = {}
    if apply_bias:
        for nm, n in (("qb", H), ("kb", H), ("vb", H), ("ob", H),
                      ("fcb", MLP), ("projb", H)):
            bias_d[nm] = nc.dram_tensor(nm, [1, n], BF16, kind="ExternalInput")
    out_d = nc.dram_tensor("out", [T, H], FP32, kind="ExternalOutput")

    with tile.TileContext(nc) as tc, ExitStack() as ctx:
        consts = ctx.enter_context(tc.tile_pool(name="consts", bufs=1))
        wpool = ctx.enter_context(tc.tile_pool(name="wpool", bufs=1))
        acts = ctx.enter_context(tc.tile_pool(name="acts", bufs=2))
        dram = ctx.enter_context(tc.tile_pool(name="dram", bufs=1, space="DRAM"))
        # PSUM budget (8 banks):
        #   big   2 x [128,1024] f32 (4 banks): k/v/q/o psums, proj accum
        #   mid   2 x [128, 512]      (2 banks): transpose staging + fc psum
        #   small 2 x [128, 260] f32 (2 banks): kv-summary accum, apply
        psum = ctx.enter_context(tc.tile_pool(name="psum", bufs=2, space="PSUM"))

        # ---- constants ----------------------------------------------------
        ident = consts.tile([128, 128], BF16)
        make_identity(nc, ident)
        ident_f8 = consts.tile([128, 128], FP8)
        nc.vector.tensor_copy(ident_f8, ident)
        eps_ln = consts.tile([128, 1], FP32)
        nc.vector.memset(eps_ln, LN_EPS)
        if apply_bias:
            ones_row = consts.tile([1, 128], BF16)
            nc.vector.memset(ones_row, 1.0)
            ones_t = consts.tile([1, BLK], BF16)
            nc.vector.memset(ones_t, 1.0)
            bias_sb = {}
            for nm, n in (("qb", H), ("kb", H), ("vb", H), ("ob", H),
                          ("fcb", MLP), ("projb", H)):
                b_t = consts.tile([1, n], BF16, name=f"{nm}_sb")
                nc.sync.dma_start(out=b_t, in_=bias_d[nm][:, :])
                bias_sb[nm] = b_t

        # resident fp8 weights (12 MB total); k/v load first (needed first)
        kw = wpool.tile([128, HC * H], FP8)
        nc.sync.dma_start(out=kw, in_=kw_d[:, :])
        vw = wpool.tile([128, HC * H], FP8)
        nc.sync.dma_start(out=vw, in_=vw_d[:, :])
        ow = wpool.tile([128, HC * H], FP8)
        nc.sync.dma_start(out=ow, in_=ow_d[:, :])
        qw = wpool.tile([128, HC * H], FP8)
        nc.sync.dma_start(out=qw, in_=qw_d[:, :])
        fcw = wpool.tile([128, MO * H], FP8)
        nc.sync.dma_start(out=fcw, in_=fcw_d[:, :])
        pjw = wpool.tile([128, MO * H], FP8)
        nc.sync.dma_start(out=pjw, in_=pjw_d[:, :])

        qw3 = qw.rearrange("p (c f) -> p c f", c=HC)   # [p, hc, FO*128]
        kw3 = kw.rearrange("p (c f) -> p c f", c=HC)   # [p, hc, H]
        vw3 = vw.rearrange("p (c f) -> p c f", c=HC)
        ow3 = ow.rearrange("p (c f) -> p c f", c=HC)
        pjw3 = pjw.rearrange("p (m f) -> p m f", m=MO)  # [p, mo, H]

        # ln1(x).T stays resident in fp8: [p, hc*T + t]
        lnxT = wpool.tile([128, HC * T], FP8)
        lnxT3 = lnxT.rearrange("p (c t) -> p c t", c=HC)

        # k/v staging for the DoubleRow kv summary: 2 manual buffers each,
        # indexed by ts-pair parity. vt is padded with a ones column per
        # head (65-stride) so one matmul produces kv and ksum together.
        kfb = [wpool.tile([128, 2 * H], FP8, name=f"kfb{i}") for i in range(2)]
        vtb = [wpool.tile([128, 2 * NH * 65], FP8, name=f"vtb{i}")
               for i in range(2)]
        for i in range(2):
            nc.vector.memset(vtb[i], 1.0)

        # kv-summary PSUM accumulators: heads 0..7 / 8..15, head h at
        # partitions (h%2)*64, cols ((h%8)//2)*65.
        # Full-bank-width tiles (cols 260:512 unused) so partition-base-64
        # writes keep each partition's flat range within one bank.
        kvs = [psum.tile([128, 512], FP32, tag="small", bufs=2, name=f"kvs{i}")
               for i in range(2)]
        for i in range(2):
            nc.vector.memset(kvs[i], 0.0)

        kvaug = consts.tile([128, 8 * 130], FP8)
        nc.vector.memset(kvaug, 0.0)

        def layernorm_to(xt, dst):
            """dst = (xt - mean) * rsqrt(var + eps), cast to dst dtype.

            Stats on vector; the 1024-wide apply runs on the scalar engine
            as Identity(x*rstd - mu*rstd)."""
            stats = acts.tile([128, 2, 6], FP32, tag="ln_stats", bufs=3)
            nc.vector.bn_stats(out=stats[:, 0, :], in_=xt[:, 0:512])
            nc.vector.bn_stats(out=stats[:, 1, :], in_=xt[:, 512:1024])
            mv = acts.tile([128, 2], FP32, tag="ln_mv", bufs=3)
            nc.vector.bn_aggr(out=mv, in_=stats)
            rstd = acts.tile([128, 1], FP32, tag="ln_rstd", bufs=3)
            nc.scalar.activation(out=rstd, in_=mv[:, 1:2], func=AF.Sqrt,
                                 bias=eps_ln, scale=1.0)
            nc.vector.reciprocal(out=rstd, in_=rstd)
            mb = acts.tile([128, 1], FP32, tag="ln_mb", bufs=3)
            nc.vector.tensor_scalar(out=mb, in0=mv[:, 0:1], scalar1=rstd[:, 0:1],
                                    scalar2=-1.0, op0=ALU.mult, op1=ALU.mult)
            nc.scalar.activation(out=dst, in_=xt, func=AF.Identity,
                                 bias=mb[:, 0:1], scale=rstd[:, 0:1])

        def transpose_chunks(src_fp8, dstT3, t0):
            """PE-transpose token-major fp8 [128,1024] into 8 chunks of
            dstT3 ([p, c, T] view, fp8), staged 4 chunks per PSUM bank.

            The fp8 transposer emits with element step 2 (16-bit lanes), so
            the staging view interleaves a dead byte per element."""
            for g in range(2):
                pt = psum.tile([128, 1024], FP8, tag="mid", bufs=2, name="pt")
                ptv = pt.rearrange("p (c t w) -> p c t w", c=4, w=2)
                for c in range(4):
                    nc.tensor.transpose(
                        ptv[:, c:c + 1, :, 0:1],
                        src_fp8[:, (g * 4 + c) * 128:(g * 4 + c + 1) * 128],
                        ident_f8)
                dst = dstT3[:, g * 4:g * 4 + 4, t0:t0 + 128] \
                    .rearrange("p c (t w) -> p c t w", w=1)
                src = ptv[:, :, :, 0:1]
                if g == 0:
                    nc.vector.tensor_copy(dst, src)
                else:
                    nc.scalar.copy(out=dst, in_=src)

        def feature_map(ps, dst, n, scale):
            """dst = elu(ps*scale)+1 = min(exp(ps*scale),1) + relu(ps*scale)."""
            e = acts.tile([128, n], BF16, tag="fm_e", bufs=3, name="fm_e")
            nc.scalar.activation(out=e, in_=ps, func=AF.Exp, scale=scale)
            r = acts.tile([128, n], BF16, tag="fm_r", bufs=2, name="fm_r")
            nc.vector.tensor_scalar(out=r, in0=ps, scalar1=0.0, scalar2=scale,
                                    op0=ALU.max, op1=ALU.mult)
            nc.vector.scalar_tensor_tensor(out=dst, in0=e, scalar=1.0, in1=r,
                                           op0=ALU.min, op1=ALU.add)

        def dr_proj(pp, lhsT3, tok0, w3, bias_t=None):
            """[128,1024] psum = x_chunkT.T @ W via fp8 DoubleRow matmuls."""
            for half in range(2):
                for c in range(HC // 2):
                    nc.tensor.matmul(
                        pp[:, half * 512:half * 512 + 512],
                        lhsT=lhsT3[:, 2 * c:2 * c + 2, tok0:tok0 + 128],
                        rhs=w3[:, 2 * c:2 * c + 2, half * 512:half * 512 + 512],
                        start=(c == 0),
                        stop=(c == HC // 2 - 1 and bias_t is None),
                        perf_mode=DR)
                if bias_t is not None:
                    nc.tensor.matmul(
                        pp[:, half * 512:half * 512 + 512], lhsT=ones_row,
                        rhs=bias_t[0:1, half * 512:half * 512 + 512],
                        start=False, stop=True)

        # ================== PASS A: ln1, k/v, kv summary ==================
        for blk in range(nblk):
            for ts in range(TS):
                xt = acts.tile([128, H], FP32, tag="xin", bufs=2)
                r0 = blk * BLK + ts * 128
                nc.gpsimd.dma_start(out=xt, in_=x_d[r0:r0 + 128, :])
                lnx = acts.tile([128, H], FP8, tag="lnx", bufs=3)
                layernorm_to(xt, lnx)
                transpose_chunks(lnx, lnxT3, r0)

            for pair in range(TS // 2):
                kf2 = kfb[pair % 2]
                vt2 = vtb[pair % 2]
                for s in range(2):
                    tok0 = blk * BLK + (pair * 2 + s) * 128
                    kp = psum.tile([128, 1024], FP32, tag="big", bufs=2,
                                   name="kp")
                    dr_proj(kp, lnxT3, tok0, kw3,
                            bias_sb["kb"] if apply_bias else None)
                    feature_map(kp, kf2[:, s * H:(s + 1) * H], H, 1.0 / SQKV)

                    vp = psum.tile([128, 1024], FP32, tag="big", bufs=2,
                                   name="vp")
                    dr_proj(vp, lnxT3, tok0, vw3,
                            bias_sb["vb"] if apply_bias else None)
                    # scaled copy into the 65-strided padded layout
                    src3 = vp.rearrange("p (h c) -> p h c", c=64)
                    dst3 = vt2.rearrange("p (g c) -> p g c", c=65)
                    d = dst3[:, s * NH:s * NH + NH, 0:64]
                    nc.vector.tensor_scalar_mul(out=d[:, 0:8, :],
                                                in0=src3[:, 0:8, :],
                                                scalar1=1.0 / SQKV)
                    nc.scalar.mul(out=d[:, 8:16, :], in_=src3[:, 8:16, :],
                                  mul=1.0 / SQKV)

                # kv summary over this 256-token pair (DoubleRow over the
                # two 128-token subtiles), accumulating in PSUM all pass.
                kf3 = kf2.rearrange("p (s f) -> p s f", s=2)
                vt3 = vt2.rearrange("p (s f) -> p s f", s=2)
                first = (blk == 0 and pair == 0)
                last = (blk == nblk - 1 and pair == TS // 2 - 1)
                # Accumulate onto the pre-zeroed PSUM with start=False
                # (plain read-modify-write); the per-bank group tracker
                # can't express 4 packed head-pair regions, so skip it.
                # Even heads (dst partitions 0:64) use DoubleRow; odd heads
                # land at partition base 64, which the DR ISA encoding
                # disallows, so they run as two plain fp8 matmuls.
                for h in range(NH):
                    tl = kvs[h // 8]
                    hh = h % 8
                    dst = tl[(hh % 2) * 64:(hh % 2) * 64 + 64,
                             (hh // 2) * 65:(hh // 2) * 65 + 65]
                    if hh % 2 == 0:
                        nc.tensor.matmul(
                            dst,
                            lhsT=kf3[:, :, h * 64:h * 64 + 64],
                            rhs=vt3[:, :, h * 65:h * 65 + 65],
                            start=False, stop=(last and hh >= 6),
                            skip_group_check=True, perf_mode=DR)
                    else:
                        for s in range(2):
                            nc.tensor.matmul(
                                dst,
                                lhsT=kf2[:, s * H + h * 64:s * H + h * 64 + 64],
                                rhs=vt2[:, s * NH * 65 + h * 65:
                                        s * NH * 65 + h * 65 + 65],
                                start=False,
                                stop=(last and hh >= 6 and s == 1),
                                skip_group_check=True)

        # ================== AllReduce of kv summary over the seq pair =====
        kv_sb = consts.tile([128, 520], BF16)
        nc.vector.tensor_copy(kv_sb[:, 0:260], kvs[0][:, 0:260])
        nc.scalar.copy(out=kv_sb[:, 260:520], in_=kvs[1][:, 0:260])
        cc_in = dram.tile([128, 520], BF16)
        cc_out = dram.tile([128, 520], BF16)
        nc.gpsimd.dma_start(out=cc_in, in_=kv_sb)
        nc.gpsimd.collective_compute(
            "AllReduce", ALU.add, replica_groups=groups,
            ins=[cc_in.opt()], outs=[cc_out.opt()])
        kvred = consts.tile([128, 520], BF16)
        nc.gpsimd.dma_start(out=kvred, in_=cc_out)

        # q projections: blocks 0/1 run while the collective is in
        # flight; later blocks pipeline in behind the attention prologue
        # and MLP tails.
        qfTs = {}

        def q_proj_stages(blk):
            qfT = acts.tile([128, FO * BLK], FP8, tag="qfT", bufs=4,
                            name="qfT")
            qfTs[blk] = qfT

            def piece(fp):
                qp = psum.tile([128, 1024], FP32, tag="big", bufs=2,
                               name="qp")
                for j in range(2):
                    fo = fp * 2 + j
                    for c in range(HC // 2):
                        nc.tensor.matmul(
                            qp[:, j * 512:j * 512 + 512],
                            lhsT=qw3[:, 2 * c:2 * c + 2,
                                     fo * 128:fo * 128 + 128],
                            rhs=lnxT3[:, 2 * c:2 * c + 2,
                                      blk * BLK:blk * BLK + BLK],
                            start=(c == 0),
                            stop=(c == HC // 2 - 1 and not apply_bias),
                            perf_mode=DR)
                    if apply_bias:
                        nc.tensor.matmul(
                            qp[:, j * 512:j * 512 + 512],
                            lhsT=bias_sb["qb"][0:1, fo * 128:fo * 128 + 128],
                            rhs=ones_t[0:1, 0:BLK], start=False, stop=True)
                feature_map(qp, qfT[:, fp * 1024:fp * 1024 + 1024], 1024,
                            1.0 / SQKV)
            return [(lambda f=fp: piece(f)) for fp in range(FO // 2)]

        for _b in range(nblk):
            for _f in q_proj_stages(_b):
                _f()

        # Block-diagonal fp8 [d, (kv|ksum)] pairs, scaled by 1/32.
        kvr3 = kvred.rearrange("p (g c) -> p g c", c=65)
        kva3 = kvaug.rearrange("p (g c) -> p g c", c=130)
        nc.vector.tensor_scalar_mul(out=kva3[0:64, :, 0:65],
                                    in0=kvr3[0:64, :, :], scalar1=SKV)
        nc.scalar.mul(out=kva3[64:128, :, 65:130], in_=kvr3[64:128, :, :],
                      mul=SKV)

        # ================== PASS B: apply, o-proj, residual, MLP ==========
        # Software-pipelined: block N's MLP (PE-dense, vector-idle) runs
        # concurrently with block N+1's attention stage (vector-heavy,
        # PE-light) by interleaving instruction emission. The attention
        # stage therefore avoids the "big" PSUM tag (held by the MLP
        # accumulators): o-proj runs as two [128,512] halves on "small".
        blk_state = {}

        def attn_stages(blk):
            """Closures for one block's attention work: per ts, A1 (apply
            + normalize), A2 (attnT transposes + o-proj + residual), A3
            (LN2 + ln2T transposes)."""
            qfT = qfTs.pop(blk)
            attnT = acts.tile([128, HC * BLK], FP8, tag="attnT", bufs=2)
            attnT3 = attnT.rearrange("p (c t) -> p c t", c=HC)
            ln2T = acts.tile([128, HC * BLK], FP8, tag="ln2T", bufs=2)
            ln2T3 = ln2T.rearrange("p (c t) -> p c t", c=HC)
            xrs = [None] * TS
            attns = [None] * TS
            blk_state[blk] = (ln2T3, xrs)

            def a1(ts):
                attn = acts.tile([128, H], FP8, tag="attn", bufs=2)
                attns[ts] = attn
                for wave in range(4):
                    pa = psum.tile([128, 512], FP32, tag="small", bufs=2,
                                   name="pa")
                    for j in range(2):
                        g = wave * 2 + j
                        nc.tensor.matmul(
                            pa[:, j * 130:j * 130 + 130],
                            lhsT=qfT[:, g * BLK + ts * 128:
                                     g * BLK + ts * 128 + 128],
                            rhs=kvaug[:, g * 130:g * 130 + 130],
                            start=(j == 0), stop=(j == 1))
                    pa3 = pa[:, 0:260].rearrange("p (g c) -> p g c", c=65)
                    rc = acts.tile([128, 4], FP32, tag="rc", bufs=4)
                    nc.vector.reciprocal(
                        out=rc,
                        in_=pa3[:, :, 64:65].rearrange("p g c -> p (g c)"))
                    rcb = bass.AP(rc.tensor, rc.offset,
                                  [rc.ap[0], (1, 4), (0, 64)])
                    dst3 = attn.rearrange("p (g c) -> p g c",
                                          c=64)[:, wave * 4:wave * 4 + 4, :]
                    nc.vector.tensor_tensor(out=dst3, in0=pa3[:, :, 0:64],
                                            in1=rcb, op=ALU.mult)

            xts = [None] * TS

            def a2a(ts):
                transpose_chunks(attns[ts], attnT3, ts * 128)
                xt = acts.tile([128, H], FP32, tag="xin", bufs=2, name="xt2")
                xts[ts] = xt
                r0 = blk * BLK + ts * 128
                nc.gpsimd.dma_start(out=xt, in_=x_d[r0:r0 + 128, :])

            def a2b(ts):
                xt = xts[ts]
                xr = acts.tile([128, H], BF16, tag="xr", bufs=2 * TS + 1,
                               name="xr")
                xrs[ts] = xr
                for half in range(2):
                    oph = psum.tile([128, 512], FP32, tag="small", bufs=2,
                                    name="oph")
                    for c in range(HC // 2):
                        nc.tensor.matmul(
                            oph,
                            lhsT=attnT3[:, 2 * c:2 * c + 2,
                                        ts * 128:ts * 128 + 128],
                            rhs=ow3[:, 2 * c:2 * c + 2,
                                    half * 512:half * 512 + 512],
                            start=(c == 0),
                            stop=(c == HC // 2 - 1 and not apply_bias),
                            perf_mode=DR)
                    if apply_bias:
                        nc.tensor.matmul(
                            oph, lhsT=ones_row,
                            rhs=bias_sb["ob"][0:1, half * 512:
                                              half * 512 + 512],
                            start=False, stop=True)
                    nc.vector.tensor_add(
                        out=xr[:, half * 512:half * 512 + 512],
                        in0=xt[:, half * 512:half * 512 + 512], in1=oph)

            ln2s = [None] * TS

            def a3a(ts):
                ln2 = acts.tile([128, H], FP8, tag="lnx", bufs=3, name="ln2")
                ln2s[ts] = ln2
                layernorm_to(xrs[ts], ln2)

            def a3b(ts):
                transpose_chunks(ln2s[ts], ln2T3, ts * 128)

            # Stagger the per-ts chains so every cross-engine dependency
            # has a couple of MLP chunks of slack before the PE needs it.
            sched = []
            for ts in range(TS):
                base = 5 * ts
                for off, fn in ((0, a1), (2, a2a), (4, a2b), (5, a3a),
                                (7, a3b)):
                    sched.append((base + off, ts, fn))
            sched.sort(key=lambda z: (z[0], z[1]))
            out = []
            for _, ts, fn in sched:
                gate = None
                if fn is a3b:
                    if ts == 1:
                        gate = (blk, 0)
                    elif ts == TS - 1:
                        gate = (blk, 1)
                out.append(((lambda t=ts, f=fn: f(t)), gate))
            return out

        from collections import deque
        pending = deque(attn_stages(0))
        gates_done = set()
        blk_pulled = []

        def pull_one():
            if pending:
                fn, gate = pending.popleft()
                fn()
                if gate is not None:
                    gates_done.add(gate)

        def drain_until(gate):
            while gate not in gates_done and pending:
                pull_one()
                if blk_pulled:
                    blk_pulled[0] += 1

        for blk in range(nblk):
            if blk + 1 < nblk:
                pending.extend(attn_stages(blk + 1))
            blk_target = len(pending)
            blk_pulled = [0]
            slot = [0]
            for th in range(2):
                drain_until((blk, th))
                ln2T3, xrs = blk_state[blk]
                pps = [psum.tile([128, 1024], FP32, tag="big", bufs=2,
                                 name=f"pps_{th}_{i}") for i in range(2)]
                for mp in range(MO // 2):
                    pfc = psum.tile([128, 512], FP32, tag="mid", bufs=2,
                                    name="pfc")
                    for j in range(2):
                        mo = mp * 2 + j
                        for c in range(HC // 2):
                            nc.tensor.matmul(
                                pfc[:, j * 256:j * 256 + 256],
                                lhsT=fcw[:, mo * 1024 + 2 * c * 128:
                                         mo * 1024 + 2 * c * 128 + 256]
                                    .rearrange("p (k m) -> p k m", k=2),
                                rhs=ln2T3[:, 2 * c:2 * c + 2,
                                          th * 256:th * 256 + 256],
                                start=(j == 0 and c == 0),
                                stop=(j == 1 and c == HC // 2 - 1
                                      and not apply_bias),
                                perf_mode=DR)
                        if apply_bias:
                            nc.tensor.matmul(
                                pfc[:, j * 256:j * 256 + 256],
                                lhsT=bias_sb["fcb"][0:1,
                                                    mo * 128:mo * 128 + 128],
                                rhs=ones_t[0:1, 0:256], start=False,
                                stop=(j == 1))
                    hT2 = acts.tile([128, 512], FP8, tag="hT", bufs=3)
                    nc.scalar.activation(out=hT2, in_=pfc,
                                         func=AF.Gelu_apprx_tanh,
                                         scale=1.0 / SFC)
                    h3 = hT2.rearrange("p (j t) -> p j t", j=2)
                    for tsl in range(2):
                        for half in range(2):
                            nc.tensor.matmul(
                                pps[tsl][:, half * 512:half * 512 + 512],
                                lhsT=h3[:, :, tsl * 128:tsl * 128 + 128],
                                rhs=pjw3[:, mp * 2:mp * 2 + 2,
                                         half * 512:half * 512 + 512],
                                start=(mp == 0),
                                stop=(mp == MO // 2 - 1 and not apply_bias),
                                perf_mode=DR)
                    # spread pipelined attention sub-stages evenly
                    slot[0] += 1
                    want = (slot[0] * blk_target) // (MO + 2)
                    while blk_pulled[0] < want and pending:
                        pull_one()
                        blk_pulled[0] += 1
                if apply_bias:
                    for tsl in range(2):
                        for half in range(2):
                            nc.tensor.matmul(
                                pps[tsl][:, half * 512:half * 512 + 512],
                                lhsT=ones_row,
                                rhs=bias_sb["projb"][0:1, half * 512:
                                                     half * 512 + 512],
                                start=False, stop=True)
                for tsl in range(2):
                    ts_ = th * 2 + tsl
                    outt = acts.tile([128, H], FP32, tag="outt", bufs=2)
                    nc.vector.scalar_tensor_tensor(
                        out=outt, in0=pps[tsl], scalar=1.0 / SPJ,
                        in1=xrs[ts_], op0=ALU.mult, op1=ALU.add)
                    r0 = blk * BLK + ts_ * 128
                    nc.sync.dma_start(out=out_d[r0:r0 + 128, :], in_=outt)
            blk_state.pop(blk)

# ======================= host side =======================================

def _prep_weights(inputs):
    """Fold LN affine params into adjacent weights; scale + cast to fp8.

    Returns (tensor_dict, apply_bias). If every effective bias is exactly
    zero (true for this model's initialization), the biases are dropped and
    the device program skips the rank-1 bias updates entirely.
    """
    f32 = lambda k: np.asarray(inputs[k], np.float32)
    bf = ml_dtypes.bfloat16
    f8 = ml_dtypes.float8_e4m3

    def to_f8(w):
        return np.clip(w, -240.0, 240.0).astype(f8)

    ln1_w, ln1_b = f32("ln1_w"), f32("ln1_b")
    ln2_w, ln2_b = f32("ln2_w"), f32("ln2_b")

    out = {}
    biases = {}

    def qkv_like(w, b, scale, q_layout):
        we = (ln1_w[:, None] * w) * scale
        be = (b + ln1_b @ w) * scale
        if q_layout:  # [p, (hc*FO+fo)*128+m]
            wd = we.reshape(HC, 128, FO, 128).transpose(1, 0, 2, 3).reshape(128, HC * H)
        else:         # [p, hc*H+m]
            wd = we.reshape(HC, 128, H).transpose(1, 0, 2).reshape(128, HC * H)
        return to_f8(wd), be.reshape(1, -1).astype(bf)

    out["qw"], biases["qb"] = qkv_like(f32("q_w"), f32("q_b"), SQKV, True)
    out["kw"], biases["kb"] = qkv_like(f32("k_w"), f32("k_b"), SQKV, False)
    out["vw"], biases["vb"] = qkv_like(f32("v_w"), f32("v_b"), SQKV, False)

    o_w, o_b = f32("o_w"), f32("o_b")
    out["ow"] = to_f8(o_w.reshape(HC, 128, H).transpose(1, 0, 2).reshape(128, HC * H))
    biases["ob"] = o_b.reshape(1, H).astype(bf)

    fc_w, fc_b = f32("fc_w"), f32("fc_b")
    fce = (ln2_w[:, None] * fc_w) * SFC
    fcbe = (fc_b + ln2_b @ fc_w) * SFC
    # [p, mo*(HC*128) + hc*128 + m]
    out["fcw"] = to_f8(fce.reshape(HC, 128, MO, 128).transpose(1, 2, 0, 3).reshape(128, MO * H))
    biases["fcb"] = fcbe.reshape(1, MLP).astype(bf)

    pj_w, pj_b = f32("proj_w"), f32("proj_b")
    out["projw"] = to_f8((pj_w * SPJ).reshape(MO, 128, H).transpose(1, 0, 2).reshape(128, MO * H))
    biases["projb"] = (pj_b * SPJ).reshape(1, H).astype(bf)

    apply_bias = any(np.any(np.asarray(b, np.float32) != 0.0)
                     for b in biases.values())
    if apply_bias:
        out.update(biases)
    return out, apply_bias


def _run(inputs, trace=False):
    from concourse.bass_utils import run_bass_kernel_spmd

    n_cores = 8
    t_core = B * S // n_cores  # 2048

    x = np.ascontiguousarray(np.asarray(inputs["x"], np.float32))
    wd, apply_bias = _prep_weights(inputs)

    nc = bacc.Bacc(None, num_devices=n_cores, target_bir_lowering=False)
    build_kernel(nc, t_core, n_cores, apply_bias=apply_bias)
    nc.compile()

    half = S // 2
    in_maps = []
    for c in range(n_cores):
        b, sh = c // 2, c % 2
        m = {"x": np.ascontiguousarray(x[b, sh * half:(sh + 1) * half, :])}
        m.update(wd)
        in_maps.append(m)

    res = run_bass_kernel_spmd(nc, in_maps, core_ids=list(range(n_cores)),
                               trace=trace)

    out = np.empty((B, S, H), np.float32)
    for c in range(n_cores):
        b, sh = c // 2, c % 2
        out[b, sh * half:(sh + 1) * half, :] = res.results[c]["out"]
    return out, res


def kernel(**inputs):
    return _run(inputs)[0]


if __name__ == "__main__":
    os.environ.setdefault("BASS_NEVER_TRACE", "1")
    import reference

    inputs = {k: np.asarray(v) for k, v in reference.setup_inputs().items()}
    got = kernel(**inputs)
    exp = np.asarray(reference.reference(**inputs))
    err = np.abs(got - exp).max() / np.abs(exp).max()
    print("Relative error:", err)


# revision 22
# speedup vs baseline: 1.0425x; 1.0356x over previous
"""Trainium2 Bass kernel for a linear-attention transformer block (fp8).

Model (see reference):
  ln1 -> q/k/v proj -> feature map elu(x)+1 -> linear attention via
  per-head kv summary [d,e] and k-sum [d] -> out proj -> residual ->
  ln2 -> MLP (gelu-tanh) -> residual.

Sharding (8 cores): token-parallel. Core c owns batch c//2, sequence half
c%2 (2048 tokens). Everything is token-local except the attention kv
summary (sum over the full sequence of a batch), which is reduced with a
pairwise AllReduce of a [128, 520] bf16 buffer (16 heads x [64, 65]
(kv | ksum), packed two heads per 128 partitions).

Device layout notes:
 - All six big GEMMs (q/k/v/o/fc/proj) run in fp8e4 with the DoubleRow
   perf mode: two 128-deep contraction tiles per matmul pass, i.e. 2x the
   bf16 matmul throughput.  Weights are pre-scaled by powers of two on the
   host so their distributions sit in fp8's sweet spot; the inverse scale
   is folded into the activation/copy op that drains each PSUM.
 - fp8 weights are small enough (12 MB total) to stay resident in SBUF,
   so nothing is re-streamed from HBM and the ln1(x).T intermediate also
   stays resident (no DRAM spill).
 - Activations are token-major [128 tokens, features]; matmuls contract
   over the partition axis, so activation tiles are PE-transposed where
   a matmul needs them feature-major, staged 4 chunks per PSUM bank with
   a single strided copy out.
 - The kv summary accumulates over the whole pass directly in PSUM
   (one long matmul accumulation group per head), not via vector adds.
 - The per-token attention normalizer is applied with a stride-0
   broadcast tensor_tensor against the reciprocal of the strided
   denominator columns read straight out of the apply PSUM.
"""

import os
import sys
from contextlib import ExitStack

import numpy as np

for _p in ("/opt/trn_rl_repo",):
    if _p not in sys.path:
        sys.path.insert(0, _p)

import ml_dtypes  # noqa: E402

import concourse.bass as bass  # noqa: E402
import concourse.tile as tile  # noqa: E402
from concourse import bacc  # noqa: E402
from concourse import mybir  # noqa: E402
from concourse.masks import make_identity  # noqa: E402

BF16 = mybir.dt.bfloat16
FP32 = mybir.dt.float32
FP8 = mybir.dt.float8e4
AF = mybir.ActivationFunctionType
ALU = mybir.AluOpType
DR = mybir.MatmulPerfMode.DoubleRow

# Model dims (fixed by the problem).
B, S, H = 4, 4096, 1024
NH, HD = 16, 64
MLP = 4096

HC = H // 128    # 8 contraction chunks over hidden dim
FO = H // 128    # 8 feature chunks (q feature-major)
MO = MLP // 128  # 32 mlp chunks
BLK = 512        # tokens per block
TS = BLK // 128  # 128-token subtiles per block

LN_EPS = 1e-5

# fp8 weight pre-scales (powers of two; inverse folded into psum drains).
SQKV = 16.0      # q/k/v weights (sigma 0.02 -> 0.32)
SFC = 16.0       # fc weights
SPJ = 64.0       # proj weights (sigma 0.004 -> 0.26)
SKV = 1.0 / 32.0  # kv-summary scale into fp8 kvaug (ksum ~4300 -> ~134)


def build_kernel(nc, t_core, n_cores, apply_bias=False):
    """Emit the per-core program."""
    T = t_core
    nblk = T // BLK
    groups = [[2 * i, 2 * i + 1] for i in range(n_cores // 2)]

    x_d = nc.dram_tensor("x", [T, H], FP32, kind="ExternalInput")
    qw_d = nc.dram_tensor("qw", [128, HC * H], FP8, kind="ExternalInput")
    kw_d = nc.dram_tensor("kw", [128, HC * H], FP8, kind="ExternalInput")
    vw_d = nc.dram_tensor("vw", [128, HC * H], FP8, kind="ExternalInput")
    ow_d = nc.dram_tensor("ow", [128, HC * H], FP8, kind="ExternalInput")
    fcw_d = nc.dram_tensor("fcw", [128, MO * H], FP8, kind="ExternalInput")
    pjw_d = nc.dram_tensor("projw", [128, MO * H], FP8, kind="ExternalInput")
    bias_d = {}
    if apply_bias:
        for nm, n in (("qb", H), ("kb", H), ("vb", H), ("ob", H),
                      ("fcb", MLP), ("projb", H)):
            bias_d[nm] = nc.dram_tensor(nm, [1, n], BF16, kind="ExternalInput")
    out_d = nc.dram_tensor("out", [T, H], FP32, kind="ExternalOutput")

    with tile.TileContext(nc) as tc, ExitStack() as ctx:
        consts = ctx.enter_context(tc.tile_pool(name="consts", bufs=1))
        wpool = ctx.enter_context(tc.tile_pool(name="wpool", bufs=1))
        acts = ctx.enter_context(tc.tile_pool(name="acts", bufs=2))
        dram = ctx.enter_context(tc.tile_pool(name="dram", bufs=1, space="DRAM"))
        # PSUM budget (8 banks):
        #   big   2 x [128,1024] f32 (4 banks): k/v/q/o psums, proj accum
        #   mid   2 x [128, 512]      (2 banks): transpose staging + fc psum
        #   small 2 x [128, 260] f32 (2 banks): kv-summary accum, apply
        psum = ctx.enter_context(tc.tile_pool(name="psum", bufs=2, space="PSUM"))

        # ---- constants ----------------------------------------------------
        ident = consts.tile([128, 128], BF16)
        make_identity(nc, ident)
        ident_f8 = consts.tile([128, 128], FP8)
        nc.vector.tensor_copy(ident_f8, ident)
        eps_ln = consts.tile([128, 1], FP32)
        nc.vector.memset(eps_ln, LN_EPS)
        if apply_bias:
            ones_row = consts.tile([1, 128], BF16)
            nc.vector.memset(ones_row, 1.0)
            ones_t = consts.tile([1, BLK], BF16)
            nc.vector.memset(ones_t, 1.0)
            bias_sb = {}
            for nm, n in (("qb", H), ("kb", H), ("vb", H), ("ob", H),
                          ("fcb", MLP), ("projb", H)):
                b_t = consts.tile([1, n], BF16, name=f"{nm}_sb")
                nc.sync.dma_start(out=b_t, in_=bias_d[nm][:, :])
                bias_sb[nm] = b_t

        # resident fp8 weights (12 MB total); k/v load first (needed first)
        kw = wpool.tile([128, HC * H], FP8)
        nc.sync.dma_start(out=kw, in_=kw_d[:, :])
        vw = wpool.tile([128, HC * H], FP8)
        nc.sync.dma_start(out=vw, in_=vw_d[:, :])
        ow = wpool.tile([128, HC * H], FP8)
        nc.sync.dma_start(out=ow, in_=ow_d[:, :])
        qw = wpool.tile([128, HC * H], FP8)
        nc.sync.dma_start(out=qw, in_=qw_d[:, :])
        fcw = wpool.tile([128, MO * H], FP8)
        nc.sync.dma_start(out=fcw, in_=fcw_d[:, :])
        pjw = wpool.tile([128, MO * H], FP8)
        nc.sync.dma_start(out=pjw, in_=pjw_d[:, :])

        qw3 = qw.rearrange("p (c f) -> p c f", c=HC)   # [p, hc, FO*128]
        kw3 = kw.rearrange("p (c f) -> p c f", c=HC)   # [p, hc, H]
        vw3 = vw.rearrange("p (c f) -> p c f", c=HC)
        ow3 = ow.rearrange("p (c f) -> p c f", c=HC)
        pjw3 = pjw.rearrange("p (m f) -> p m f", m=MO)  # [p, mo, H]

        # ln1(x).T stays resident in fp8: [p, hc*T + t]
        lnxT = wpool.tile([128, HC * T], FP8)
        lnxT3 = lnxT.rearrange("p (c t) -> p c t", c=HC)

        # k/v staging for the DoubleRow kv summary: 2 manual buffers each,
        # indexed by ts-pair parity. vt is padded with a ones column per
        # head (65-stride) so one matmul produces kv and ksum together.
        kfb = [wpool.tile([128, 2 * H], FP8, name=f"kfb{i}") for i in range(2)]
        vtb = [wpool.tile([128, 2 * NH * 65], FP8, name=f"vtb{i}")
               for i in range(2)]
        for i in range(2):
            nc.vector.memset(vtb[i], 1.0)

        # kv-summary PSUM accumulators: heads 0..7 / 8..15, head h at
        # partitions (h%2)*64, cols ((h%8)//2)*65.
        # Full-bank-width tiles (cols 260:512 unused) so partition-base-64
        # writes keep each partition's flat range within one bank.
        kvs = [psum.tile([128, 512], FP32, tag="small", bufs=2, name=f"kvs{i}")
               for i in range(2)]
        for i in range(2):
            nc.vector.memset(kvs[i], 0.0)

        kvaug = consts.tile([128, 8 * 130], FP8)
        nc.vector.memset(kvaug, 0.0)

        def layernorm_to(xt, dst):
            """dst = (xt - mean) * rsqrt(var + eps), cast to dst dtype.

            Stats on vector; the 1024-wide apply runs on the scalar engine
            as Identity(x*rstd - mu*rstd)."""
            stats = acts.tile([128, 2, 6], FP32, tag="ln_stats", bufs=3)
            nc.vector.bn_stats(out=stats[:, 0, :], in_=xt[:, 0:512])
            nc.vector.bn_stats(out=stats[:, 1, :], in_=xt[:, 512:1024])
            mv = acts.tile([128, 2], FP32, tag="ln_mv", bufs=3)
            nc.vector.bn_aggr(out=mv, in_=stats)
            rstd = acts.tile([128, 1], FP32, tag="ln_rstd", bufs=3)
            nc.scalar.activation(out=rstd, in_=mv[:, 1:2], func=AF.Sqrt,
                                 bias=eps_ln, scale=1.0)
            nc.vector.reciprocal(out=rstd, in_=rstd)
            mb = acts.tile([128, 1], FP32, tag="ln_mb", bufs=3)
            nc.vector.tensor_scalar(out=mb, in0=mv[:, 0:1], scalar1=rstd[:, 0:1],
                                    scalar2=-1.0, op0=ALU.mult, op1=ALU.mult)
            nc.scalar.activation(out=dst, in_=xt, func=AF.Identity,
                                 bias=mb[:, 0:1], scale=rstd[:, 0:1])

        def transpose_chunks(src_fp8, dstT3, t0):
            """PE-transpose token-major fp8 [128,1024] into 8 chunks of
            dstT3 ([p, c, T] view, fp8), staged 4 chunks per PSUM bank.

            The fp8 transposer emits with element step 2 (16-bit lanes), so
            the staging view interleaves a dead byte per element."""
            for g in range(2):
                pt = psum.tile([128, 1024], FP8, tag="mid", bufs=2, name="pt")
                ptv = pt.rearrange("p (c t w) -> p c t w", c=4, w=2)
                for c in range(4):
                    nc.tensor.transpose(
                        ptv[:, c:c + 1, :, 0:1],
                        src_fp8[:, (g * 4 + c) * 128:(g * 4 + c + 1) * 128],
                        ident_f8)
                dst = dstT3[:, g * 4:g * 4 + 4, t0:t0 + 128] \
                    .rearrange("p c (t w) -> p c t w", w=1)
                src = ptv[:, :, :, 0:1]
                if g == 0:
                    nc.vector.tensor_copy(dst, src)
                else:
                    nc.scalar.copy(out=dst, in_=src)

        def feature_map(ps, dst, n, scale):
            """dst = elu(ps*scale)+1 = min(exp(ps*scale),1) + relu(ps*scale)."""
            e = acts.tile([128, n], BF16, tag="fm_e", bufs=3, name="fm_e")
            nc.scalar.activation(out=e, in_=ps, func=AF.Exp, scale=scale)
            r = acts.tile([128, n], BF16, tag="fm_r", bufs=2, name="fm_r")
            nc.vector.tensor_scalar(out=r, in0=ps, scalar1=0.0, scalar2=scale,
                                    op0=ALU.max, op1=ALU.mult)
            nc.vector.scalar_tensor_tensor(out=dst, in0=e, scalar=1.0, in1=r,
                                           op0=ALU.min, op1=ALU.add)

        def dr_proj(pp, lhsT3, tok0, w3, bias_t=None):
            """[128,1024] psum = x_chunkT.T @ W via fp8 DoubleRow matmuls."""
            for half in range(2):
                for c in range(HC // 2):
                    nc.tensor.matmul(
                        pp[:, half * 512:half * 512 + 512],
                        lhsT=lhsT3[:, 2 * c:2 * c + 2, tok0:tok0 + 128],
                        rhs=w3[:, 2 * c:2 * c + 2, half * 512:half * 512 + 512],
                        start=(c == 0),
                        stop=(c == HC // 2 - 1 and bias_t is None),
                        perf_mode=DR)
                if bias_t is not None:
                    nc.tensor.matmul(
                        pp[:, half * 512:half * 512 + 512], lhsT=ones_row,
                        rhs=bias_t[0:1, half * 512:half * 512 + 512],
                        start=False, stop=True)

        # ================== PASS A: ln1, k/v, kv summary ==================
        for blk in range(nblk):
            for ts in range(TS):
                xt = acts.tile([128, H], FP32, tag="xin", bufs=2)
                r0 = blk * BLK + ts * 128
                nc.gpsimd.dma_start(out=xt, in_=x_d[r0:r0 + 128, :])
                lnx = acts.tile([128, H], FP8, tag="lnx", bufs=3)
                layernorm_to(xt, lnx)
                transpose_chunks(lnx, lnxT3, r0)

            for pair in range(TS // 2):
                kf2 = kfb[pair % 2]
                vt2 = vtb[pair % 2]
                for s in range(2):
                    tok0 = blk * BLK + (pair * 2 + s) * 128
                    kp = psum.tile([128, 1024], FP32, tag="big", bufs=2,
                                   name="kp")
                    dr_proj(kp, lnxT3, tok0, kw3,
                            bias_sb["kb"] if apply_bias else None)
                    feature_map(kp, kf2[:, s * H:(s + 1) * H], H, 1.0 / SQKV)

                    vp = psum.tile([128, 1024], FP32, tag="big", bufs=2,
                                   name="vp")
                    dr_proj(vp, lnxT3, tok0, vw3,
                            bias_sb["vb"] if apply_bias else None)
                    # scaled copy into the 65-strided padded layout
                    src3 = vp.rearrange("p (h c) -> p h c", c=64)
                    dst3 = vt2.rearrange("p (g c) -> p g c", c=65)
                    d = dst3[:, s * NH:s * NH + NH, 0:64]
                    nc.vector.tensor_scalar_mul(out=d[:, 0:8, :],
                                                in0=src3[:, 0:8, :],
                                                scalar1=1.0 / SQKV)
                    nc.scalar.mul(out=d[:, 8:16, :], in_=src3[:, 8:16, :],
                                  mul=1.0 / SQKV)

                # kv summary over this 256-token pair (DoubleRow over the
                # two 128-token subtiles), accumulating in PSUM all pass.
                kf3 = kf2.rearrange("p (s f) -> p s f", s=2)
                vt3 = vt2.rearrange("p (s f) -> p s f", s=2)
                first = (blk == 0 and pair == 0)
                last = (blk == nblk - 1 and pair == TS // 2 - 1)
                # Accumulate onto the pre-zeroed PSUM with start=False
                # (plain read-modify-write); the per-bank group tracker
                # can't express 4 packed head-pair regions, so skip it.
                # Even heads (dst partitions 0:64) use DoubleRow; odd heads
                # land at partition base 64, which the DR ISA encoding
                # disallows, so they run as two plain fp8 matmuls.
                for h in range(NH):
                    tl = kvs[h // 8]
                    hh = h % 8
                    dst = tl[(hh % 2) * 64:(hh % 2) * 64 + 64,
                             (hh // 2) * 65:(hh // 2) * 65 + 65]
                    if hh % 2 == 0:
                        nc.tensor.matmul(
                            dst,
                            lhsT=kf3[:, :, h * 64:h * 64 + 64],
                            rhs=vt3[:, :, h * 65:h * 65 + 65],
                            start=False, stop=(last and hh >= 6),
                            skip_group_check=True, perf_mode=DR)
                    else:
                        for s in range(2):
                            nc.tensor.matmul(
                                dst,
                                lhsT=kf2[:, s * H + h * 64:s * H + h * 64 + 64],
                                rhs=vt2[:, s * NH * 65 + h * 65:
                                        s * NH * 65 + h * 65 + 65],
                                start=False,
                                stop=(last and hh >= 6 and s == 1),
                                skip_group_check=True)

        # ================== AllReduce of kv summary over the seq pair =====
        kv_sb = consts.tile([128, 520], BF16)
        nc.vector.tensor_copy(kv_sb[:, 0:260], kvs[0][:, 0:260])
        nc.scalar.copy(out=kv_sb[:, 260:520], in_=kvs[1][:, 0:260])
        cc_in = dram.tile([128, 520], BF16)
        cc_out = dram.tile([128, 520], BF16)
        nc.gpsimd.dma_start(out=cc_in, in_=kv_sb)
        nc.gpsimd.collective_compute(
            "AllReduce", ALU.add, replica_groups=groups,
            ins=[cc_in.opt()], outs=[cc_out.opt()])
        kvred = consts.tile([128, 520], BF16)
        nc.gpsimd.dma_start(out=kvred, in_=cc_out)

        # q projections: blocks 0/1 run while the collective is in
        # flight; later blocks pipeline in behind the attention prologue
        # and MLP tails.
        qfTs = {}

        def q_proj_stages(blk):
            qfT = acts.tile([128, FO * BLK], FP8, tag="qfT", bufs=4,
                            name="qfT")
            qfTs[blk] = qfT

            def piece(fp):
                qp = psum.tile([128, 1024], FP32, tag="big", bufs=2,
                               name="qp")
                for j in range(2):
                    fo = fp * 2 + j
                    for c in range(HC // 2):
                        nc.tensor.matmul(
                            qp[:, j * 512:j * 512 + 512],
                            lhsT=qw3[:, 2 * c:2 * c + 2,
                                     fo * 128:fo * 128 + 128],
                            rhs=lnxT3[:, 2 * c:2 * c + 2,
                                      blk * BLK:blk * BLK + BLK],
                            start=(c == 0),
                            stop=(c == HC // 2 - 1 and not apply_bias),
                            perf_mode=DR)
                    if apply_bias:
                        nc.tensor.matmul(
                            qp[:, j * 512:j * 512 + 512],
                            lhsT=bias_sb["qb"][0:1, fo * 128:fo * 128 + 128],
                            rhs=ones_t[0:1, 0:BLK], start=False, stop=True)
                feature_map(qp, qfT[:, fp * 1024:fp * 1024 + 1024], 1024,
                            1.0 / SQKV)
            return [(lambda f=fp: piece(f)) for fp in range(FO // 2)]

        for _b in range(min(2, nblk)):
            for _f in q_proj_stages(_b):
                _f()

        # Block-diagonal fp8 [d, (kv|ksum)] pairs, scaled by 1/32.
        kvr3 = kvred.rearrange("p (g c) -> p g c", c=65)
        kva3 = kvaug.rearrange("p (g c) -> p g c", c=130)
        nc.vector.tensor_scalar_mul(out=kva3[0:64, :, 0:65],
                                    in0=kvr3[0:64, :, :], scalar1=SKV)
        nc.scalar.mul(out=kva3[64:128, :, 65:130], in_=kvr3[64:128, :, :],
                      mul=SKV)

        # ================== PASS B: apply, o-proj, residual, MLP ==========
        # Software-pipelined: block N's MLP (PE-dense, vector-idle) runs
        # concurrently with block N+1's attention stage (vector-heavy,
        # PE-light) by interleaving instruction emission. The attention
        # stage therefore avoids the "big" PSUM tag (held by the MLP
        # accumulators): o-proj runs as two [128,512] halves on "small".
        blk_state = {}

        def attn_stages(blk):
            """Closures for one block's attention work: per ts, A1 (apply
            + normalize), A2 (attnT transposes + o-proj + residual), A3
            (LN2 + ln2T transposes)."""
            qfT = qfTs.pop(blk)
            attnT = acts.tile([128, HC * BLK], FP8, tag="attnT", bufs=2)
            attnT3 = attnT.rearrange("p (c t) -> p c t", c=HC)
            ln2T = acts.tile([128, HC * BLK], FP8, tag="ln2T", bufs=2)
            ln2T3 = ln2T.rearrange("p (c t) -> p c t", c=HC)
            xrs = [None] * TS
            attns = [None] * TS
            blk_state[blk] = (ln2T3, xrs)

            def a1(ts):
                attn = acts.tile([128, H], FP8, tag="attn", bufs=2)
                attns[ts] = attn
                for wave in range(4):
                    pa = psum.tile([128, 512], FP32, tag="small", bufs=2,
                                   name="pa")
                    for j in range(2):
                        g = wave * 2 + j
                        nc.tensor.matmul(
                            pa[:, j * 130:j * 130 + 130],
                            lhsT=qfT[:, g * BLK + ts * 128:
                                     g * BLK + ts * 128 + 128],
                            rhs=kvaug[:, g * 130:g * 130 + 130],
                            start=(j == 0), stop=(j == 1))
                    pa3 = pa[:, 0:260].rearrange("p (g c) -> p g c", c=65)
                    rc = acts.tile([128, 4], FP32, tag="rc", bufs=4)
                    nc.vector.reciprocal(
                        out=rc,
                        in_=pa3[:, :, 64:65].rearrange("p g c -> p (g c)"))
                    rcb = bass.AP(rc.tensor, rc.offset,
                                  [rc.ap[0], (1, 4), (0, 64)])
                    dst3 = attn.rearrange("p (g c) -> p g c",
                                          c=64)[:, wave * 4:wave * 4 + 4, :]
                    nc.vector.tensor_tensor(out=dst3, in0=pa3[:, :, 0:64],
                                            in1=rcb, op=ALU.mult)

            xts = [None] * TS

            def a2a(ts):
                transpose_chunks(attns[ts], attnT3, ts * 128)
                xt = acts.tile([128, H], FP32, tag="xin", bufs=2, name="xt2")
                xts[ts] = xt
                r0 = blk * BLK + ts * 128
                nc.gpsimd.dma_start(out=xt, in_=x_d[r0:r0 + 128, :])

            def a2b(ts):
                xt = xts[ts]
                xr = acts.tile([128, H], BF16, tag="xr", bufs=2 * TS + 1,
                               name="xr")
                xrs[ts] = xr
                for half in range(2):
                    oph = psum.tile([128, 512], FP32, tag="small", bufs=2,
                                    name="oph")
                    for c in range(HC // 2):
                        nc.tensor.matmul(
                            oph,
                            lhsT=attnT3[:, 2 * c:2 * c + 2,
                                        ts * 128:ts * 128 + 128],
                            rhs=ow3[:, 2 * c:2 * c + 2,
                                    half * 512:half * 512 + 512],
                            start=(c == 0),
                            stop=(c == HC // 2 - 1 and not apply_bias),
                            perf_mode=DR)
                    if apply_bias:
                        nc.tensor.matmul(
                            oph, lhsT=ones_row,
                            rhs=bias_sb["ob"][0:1, half * 512:
                                              half * 512 + 512],
                            start=False, stop=True)
                    nc.vector.tensor_add(
                        out=xr[:, half * 512:half * 512 + 512],
                        in0=xt[:, half * 512:half * 512 + 512], in1=oph)

            ln2s = [None] * TS

            def a3a(ts):
                ln2 = acts.tile([128, H], FP8, tag="lnx", bufs=3, name="ln2")
                ln2s[ts] = ln2
                layernorm_to(xrs[ts], ln2)

            def a3b(ts):
                transpose_chunks(ln2s[ts], ln2T3, ts * 128)

            # Stagger the per-ts chains so every cross-engine dependency
            # has a couple of MLP chunks of slack before the PE needs it.
            sched = []
            for ts in range(TS):
                base = 5 * ts
                for off, fn in ((0, a1), (2, a2a), (4, a2b), (5, a3a),
                                (7, a3b)):
                    sched.append((base + off, ts, fn))
            sched.sort(key=lambda z: (z[0], z[1]))
            out = []
            for _, ts, fn in sched:
                gate = None
                if fn is a3b:
                    if ts == 1:
                        gate = (blk, 0)
                    elif ts == TS - 1:
                        gate = (blk, 1)
                out.append(((lambda t=ts, f=fn: f(t)), gate))
            return out

        # Prologue: block 0's attention interleaved with q_proj(2,3)
        # pieces, which also keep the PE busy while the collective lands.
        pro = [f for f, _ in attn_stages(0)]
        qrest = []
        for _b in range(2, nblk):
            qrest.extend(q_proj_stages(_b))
        order = []
        for i in range(max(len(pro), len(qrest))):
            if i < len(qrest):
                order.append(qrest[i])
            if i < len(pro):
                order.append(pro[i])
        for f in order:
            f()

        for blk in range(nblk):
            ln2T3, xrs = blk_state.pop(blk)
            stages = [f for f, _ in attn_stages(blk + 1)] \
                if blk + 1 < nblk else []
            n_slots = MO + 2
            emitted = 0
            slot = 0
            for th in range(2):
                pps = [psum.tile([128, 1024], FP32, tag="big", bufs=2,
                                 name=f"pps_{th}_{i}") for i in range(2)]
                for mp in range(MO // 2):
                    pfc = psum.tile([128, 512], FP32, tag="mid", bufs=2,
                                    name="pfc")
                    for j in range(2):
                        mo = mp * 2 + j
                        for c in range(HC // 2):
                            nc.tensor.matmul(
                                pfc[:, j * 256:j * 256 + 256],
                                lhsT=fcw[:, mo * 1024 + 2 * c * 128:
                                         mo * 1024 + 2 * c * 128 + 256]
                                    .rearrange("p (k m) -> p k m", k=2),
                                rhs=ln2T3[:, 2 * c:2 * c + 2,
                                          th * 256:th * 256 + 256],
                                start=(j == 0 and c == 0),
                                stop=(j == 1 and c == HC // 2 - 1
                                      and not apply_bias),
                                perf_mode=DR)
                        if apply_bias:
                            nc.tensor.matmul(
                                pfc[:, j * 256:j * 256 + 256],
                                lhsT=bias_sb["fcb"][0:1,
                                                    mo * 128:mo * 128 + 128],
                                rhs=ones_t[0:1, 0:256], start=False,
                                stop=(j == 1))
                    hT2 = acts.tile([128, 512], FP8, tag="hT", bufs=3)
                    nc.scalar.activation(out=hT2, in_=pfc,
                                         func=AF.Gelu_apprx_tanh,
                                         scale=1.0 / SFC)
                    h3 = hT2.rearrange("p (j t) -> p j t", j=2)
                    for tsl in range(2):
                        for half in range(2):
                            nc.tensor.matmul(
                                pps[tsl][:, half * 512:half * 512 + 512],
                                lhsT=h3[:, :, tsl * 128:tsl * 128 + 128],
                                rhs=pjw3[:, mp * 2:mp * 2 + 2,
                                         half * 512:half * 512 + 512],
                                start=(mp == 0),
                                stop=(mp == MO // 2 - 1 and not apply_bias),
                                perf_mode=DR)
                    # spread pipelined attention sub-stages evenly
                    slot += 1
                    want = (slot * len(stages)) // n_slots if stages else 0
                    while emitted < want:
                        stages[emitted]()
                        emitted += 1
                if apply_bias:
                    for tsl in range(2):
                        for half in range(2):
                            nc.tensor.matmul(
                                pps[tsl][:, half * 512:half * 512 + 512],
                                lhsT=ones_row,
                                rhs=bias_sb["projb"][0:1, half * 512:
                                                     half * 512 + 512],
                                start=False, stop=True)
                for tsl in range(2):
                    ts_ = th * 2 + tsl
                    outt = acts.tile([128, H], FP32, tag="outt", bufs=2)
                    nc.vector.scalar_tensor_tensor(
                        out=outt, in0=pps[tsl], scalar=1.0 / SPJ,
                        in1=xrs[ts_], op0=ALU.mult, op1=ALU.add)
                    r0 = blk * BLK + ts_ * 128
                    nc.sync.dma_start(out=out_d[r0:r0 + 128, :], in_=outt)
            while emitted < len(stages):
                stages[emitted]()
                emitted += 1

# ======================= host side =======================================

def _prep_weights(inputs):
    """Fold LN affine params into adjacent weights; scale + cast to fp8.

    Returns (tensor_dict, apply_bias). If every effective bias is exactly
    zero (true for this model's initialization), the biases are dropped and
    the device program skips the rank-1 bias updates entirely.
    """
    f32 = lambda k: np.asarray(inputs[k], np.float32)
    bf = ml_dtypes.bfloat16
    f8 = ml_dtypes.float8_e4m3

    def to_f8(w):
        return np.clip(w, -240.0, 240.0).astype(f8)

    ln1_w, ln1_b = f32("ln1_w"), f32("ln1_b")
    ln2_w, ln2_b = f32("ln2_w"), f32("ln2_b")

    out = {}
    biases = {}

    def qkv_like(w, b, scale, q_layout):
        we = (ln1_w[:, None] * w) * scale
        be = (b + ln1_b @ w) * scale
        if q_layout:  # [p, (hc*FO+fo)*128+m]
            wd = we.reshape(HC, 128, FO, 128).transpose(1, 0, 2, 3).reshape(128, HC * H)
        else:         # [p, hc*H+m]
            wd = we.reshape(HC, 128, H).transpose(1, 0, 2).reshape(128, HC * H)
        return to_f8(wd), be.reshape(1, -1).astype(bf)

    out["qw"], biases["qb"] = qkv_like(f32("q_w"), f32("q_b"), SQKV, True)
    out["kw"], biases["kb"] = qkv_like(f32("k_w"), f32("k_b"), SQKV, False)
    out["vw"], biases["vb"] = qkv_like(f32("v_w"), f32("v_b"), SQKV, False)

    o_w, o_b = f32("o_w"), f32("o_b")
    out["ow"] = to_f8(o_w.reshape(HC, 128, H).transpose(1, 0, 2).reshape(128, HC * H))
    biases["ob"] = o_b.reshape(1, H).astype(bf)

    fc_w, fc_b = f32("fc_w"), f32("fc_b")
    fce = (ln2_w[:, None] * fc_w) * SFC
    fcbe = (fc_b + ln2_b @ fc_w) * SFC
    # [p, mo*(HC*128) + hc*128 + m]
    out["fcw"] = to_f8(fce.reshape(HC, 128, MO, 128).transpose(1, 2, 0, 3).reshape(128, MO * H))
    biases["fcb"] = fcbe.reshape(1, MLP).astype(bf)

    pj_w, pj_b = f32("proj_w"), f32("proj_b")
    out["projw"] = to_f8((pj_w * SPJ).reshape(MO, 128, H).transpose(1, 0, 2).reshape(128, MO * H))
    biases["projb"] = (pj_b * SPJ).reshape(1, H).astype(bf)

    apply_bias = any(np.any(np.asarray(b, np.float32) != 0.0)
                     for b in biases.values())
    if apply_bias:
        out.update(biases)
    return out, apply_bias


def _run(inputs, trace=False):
    from concourse.bass_utils import run_bass_kernel_spmd

    n_cores = 8
    t_core = B * S // n_cores  # 2048

    x = np.ascontiguousarray(np.asarray(inputs["x"], np.float32))
    wd, apply_bias = _prep_weights(inputs)

    nc = bacc.Bacc(None, num_devices=n_cores, target_bir_lowering=False)
    build_kernel(nc, t_core, n_cores, apply_bias=apply_bias)
    nc.compile()

    half = S // 2
    in_maps = []
    for c in range(n_cores):
        b, sh = c // 2, c % 2
        m = {"x": np.ascontiguousarray(x[b, sh * half:(sh + 1) * half, :])}
        m.update(wd)
        in_maps.append(m)

    res = run_bass_kernel_spmd(nc, in_maps, core_ids=list(range(n_cores)),
                               trace=trace)

    out = np.empty((B, S, H), np.float32)
    for c in range(n_cores):
        b, sh = c // 2, c % 2
        out[b, sh * half:(sh + 1) * half, :] = res.results[c]["out"]
    return out, res


def kernel(**inputs):
    return _run(inputs)[0]


if __name__ == "__main__":
    os.environ.setdefault("BASS_NEVER_TRACE", "1")
    import reference

    inputs = {k: np.asarray(v) for k, v in reference.setup_inputs().items()}
    got = kernel(**inputs)
    exp = np.asarray(reference.reference(**inputs))
    err = np.abs(got - exp).max() / np.abs(exp).max()
    print("Relative error:", err)
